# revision 6
# baseline (speedup 1.0000x reference)
"""GAT (5-layer, dense-adjacency) Trainium2 kernel v2, 8 NeuronCores.

Key ideas vs v1:
- factored exp: exp(leaky(s_i+d_j)) == max(e^{s_i}e^{d_j}, e^{.1s_i}e^{.1d_j})
  (exp is monotone, leaky(x)=max(x,.1x)); dividing by e^{.1 s_i} (cancels in
  softmax) gives p[j,i] = max(w_i*v_j, v'_j)*m_ij with w=e^{.9s}, v=e^d,
  v'=e^{.1d}. ONE custom bf16 DVE op per (jt,head) replaces leaky+mask+exp.
- binary multiplicative bf16 mask, host-transposed (no on-chip transposes).
- Wh transported through AllGather as single bf16 (+ d as bf16 hi/lo),
  already padded with the ones column used for softmax denominators.
- attention matmul: single bf16 matmul per (jt,head), lhsT [128, dh+1].
- AllGather split per head-group; second half overlaps first group's attention.
"""

import numpy as np

import concourse.bacc as bacc
import concourse.mybir as mybir
import concourse.tile as tile
from concourse.bass_utils import run_bass_kernel_spmd

import concourse.dve_ops as dve_ops
from concourse.dve_spec import Spec, Src0, Src1, C0, Zero, One, C1, maxx, minn, lower
from concourse.dve_spec import _has_src1 as _spec_has_src1
from concourse.dve_uop import DveOpSpec

import ml_dtypes

BF16 = ml_dtypes.bfloat16

dt = mybir.dt
AF = mybir.ActivationFunctionType

# ---------------------------------------------------------------- constants
N = 4096
NCORE = 8
ROWS = N // NCORE  # 512
P = 128
JT = N // P  # 32
# (fin, fout, h, dh, elu_after)
CFG = [
    (256, 128, 8, 16, True),
    (128, 64, 8, 8, True),
    (64, 32, 4, 8, True),
    (32, 16, 1, 16, False),
    (16, 8, 1, 8, False),
]
D_LO = True  # transport d as bf16 hi+lo (False: single bf16)

# ---------------------------------------------------------------- custom ops
GATP = dve_ops.DveOp(
    "GATP",
    Spec(
        body=maxx(Src0 * C0, C1) * Src1,
        reference=lambda in0, in1, s0, s1, imm2: (
            np.maximum(in0 * s0, s1) * in1
        ).astype(np.float32),
    ),
    subdim=False,
    uops_sha={},
)

# elu(x) = max(x,0) - 1 + exp(min(x,0)); exp(min(x,0)) == min(e^x, 1)
ELUC = dve_ops.DveOp(
    "ELUC",
    Spec(
        body=maxx(Src0, Zero) + (minn(Src1, One) + C0),
        reference=lambda in0, in1, s0, s1, imm2: (
            np.maximum(in0, 0.0) + np.minimum(in1, 1.0) + s0
        ).astype(np.float32),
    ),
    subdim=False,
    uops_sha={},
)


def _register_custom_op(op):
    if op.name in dve_ops._SUB_OPCODE_FOR_NAME:
        return
    idx = dve_ops._CUSTOM_DVE_ROW_BASE + len(dve_ops.OPS)
    assert idx < 0x20
    dve_ops.OPS.append(op)
    dve_ops.CUSTOM_DVE_SPECS[op.name] = op.spec
    dve_ops._SUB_OPCODE_FOR_NAME[op.name] = idx
    shas = {}
    for ver in ("v3", "v4"):
        try:
            s = DveOpSpec(
                name=op.name,
                opcode=idx,
                uops=lower(op.spec, ver=ver),
                rd1_en=_spec_has_src1(op.spec),
            )
            shas[ver] = s.sha(ver)
        except Exception:
            pass
    object.__setattr__(op, "uops_sha", shas)


_register_custom_op(GATP)
_register_custom_op(ELUC)


def _groups(h):
    """Head groups: split in two for h>1 (AG overlap), else single group."""
    if h == 1:
        return [(0, 1)]
    hA = h // 2
    return [(0, hA), (hA, h)]


def _gcols(h0, h1, dh):
    """Payload columns for heads [h0,h1): packed (dh+2)-stride blocks
    (values 0..dh-1, d hi/lo at dh/dh+1)."""
    g = h1 - h0
    return g * (dh + 2)


# ---------------------------------------------------------------- builder
def build_kernel():
    nc = bacc.Bacc("TRN2", target_bir_lowering=False, debug=False, num_swdge_queues=4)

    maskTd = nc.dram_tensor("maskTd", [JT, P, ROWS], dt.bfloat16, kind="ExternalInput")
    x0T_own = nc.dram_tensor("x0T_own", [256, ROWS], dt.float32, kind="ExternalInput")
    wall_dram = {}
    ws_dram = {}
    for li, (fin, fout, h, dh, _e) in enumerate(CFG, start=1):
        wall_dram[li] = nc.dram_tensor(
            f"wall{li}", [fin, h * dh + h], dt.float32, kind="ExternalInput"
        )
        ws_dram[li] = nc.dram_tensor(f"ws{li}", [fin, h], dt.float32, kind="ExternalInput")

    pool_out = nc.dram_tensor("pool_part", [8, 1], dt.float32, kind="ExternalOutput")
    import os as _os
    debug_taps = bool(_os.environ.get("DEBUG_TAPS"))
    dbg_x = {}
    if debug_taps:
        for _li, (_f, _fo, _h, _dh, _e) in enumerate(CFG, start=1):
            dbg_x[_li] = nc.dram_tensor(f"dbg_x{_li}", [_fo, ROWS], dt.float32, kind="ExternalOutput")
        dbg_d = nc.dram_tensor("dbg_d1", [P, JT, 8], dt.float32, kind="ExternalOutput")
        dbg_den = nc.dram_tensor("dbg_den1", [33, ROWS], dt.float32, kind="ExternalOutput")

    with tile.TileContext(nc) as tc:
        with (
            tc.tile_pool(name="persist", bufs=1) as persist,
            tc.tile_pool(name="dram", bufs=1, space="DRAM") as drampool,
            tc.tile_pool(name="xpool", bufs=2) as xpool,
            tc.tile_pool(name="layerbuf", bufs=1) as layerbuf,
            tc.tile_pool(name="work", bufs=2) as work,
            tc.tile_pool(name="small", bufs=3) as small,
            tc.tile_pool(name="whps", bufs=2, space="PSUM") as whps,
            tc.tile_pool(name="sps", bufs=2, space="PSUM") as sps,
            tc.tile_pool(name="attps", bufs=4, space="PSUM") as attps,
        ):
            # ---------------- persistent tiles
            maskT = persist.tile([P, JT, ROWS], dt.bfloat16, tag="maskT")
            for jt in range(JT):
                eng = nc.sync if jt % 4 == 0 else nc.gpsimd
                eng.dma_start(maskT[:, jt, :], maskTd[jt])

            wall_sb = {}
            ws_sb = {}
            for li, (fin, fout, h, dh, _e) in enumerate(CFG, start=1):
                cwa = h * dh + h
                nft = (fin + P - 1) // P
                wall_sb[li] = []
                ws_sb[li] = []
                for ft in range(nft):
                    fr = min(P, fin - ft * P)
                    wt = persist.tile([fr, cwa], dt.float32, tag=f"wall{li}_{ft}", name=f"wall{li}_{ft}")
                    nc.sync.dma_start(wt[:], wall_dram[li][ft * P : ft * P + fr, :])
                    wall_sb[li].append(wt)
                    st = persist.tile([fr, h], dt.float32, tag=f"ws{li}_{ft}", name=f"ws{li}_{ft}")
                    nc.sync.dma_start(st[:], ws_dram[li][ft * P : ft * P + fr, :])
                    ws_sb[li].append(st)

            # ---------------- L1 own activations
            xcur = []
            for ft in range(2):
                xt = xpool.tile([P, ROWS], dt.float32, tag="xT", name=f"x0_{ft}")
                nc.sync.dma_start(xt[:], x0T_own[ft * P : (ft + 1) * P, :])
                xcur.append(xt)

            for li, (fin, fout, h, dh, elu) in enumerate(CFG, start=1):
                nft = (fin + P - 1) // P
                hdh = h * dh
                cwa = hdh + h
                groups = _groups(h)
                goff = [0]
                for (g0, g1) in groups:
                    goff.append(goff[-1] + _gcols(g0, g1, dh))
                cwp = goff[-1]
                is_last = li == len(CFG)

                # ---- (A) own-block Wh + d -> packed (dh+2)-stride payload
                own_pay = work.tile([P, 4, cwp], dt.bfloat16, tag="own_pay")
                for k in range(4):
                    pw = whps.tile([P, cwa], dt.float32, tag="pw")
                    for ft in range(nft):
                        fr = min(P, fin - ft * P)
                        nc.tensor.matmul(
                            pw[:],
                            xcur[ft][0:fr, k * P : (k + 1) * P],
                            wall_sb[li][ft][:],
                            start=(ft == 0),
                            stop=(ft == nft - 1),
                        )
                    kv = own_pay[:, k, :].rearrange("p (a b) -> p a b", b=dh + 2)
                    nc.scalar.copy(
                        kv[:, :, 0:dh],
                        pw[:, 0:hdh].rearrange("p (a b) -> p a b", b=dh),
                    )
                    nc.scalar.copy(
                        kv[:, :, dh : dh + 1],
                        pw[:, hdh : hdh + h].rearrange("p (a b) -> p a b", b=1),
                    )
                    nc.vector.tensor_sub(
                        kv[:, :, dh + 1 : dh + 2],
                        pw[:, hdh : hdh + h].rearrange("p (a b) -> p a b", b=1),
                        kv[:, :, dh : dh + 1],
                    )

                # ---- (C) single AllGather per layer (emitted before (B) so
                # the wire starts while s/w_rep compute)
                agi = drampool.tile(
                    [4 * P, cwp], dt.bfloat16, tag=f"agin{li}", name=f"agin{li}"
                )
                ago = drampool.tile(
                    [NCORE, 4 * P, cwp],
                    dt.bfloat16,
                    tag=f"agout{li}",
                    name=f"agout{li}",
                    addr_space="Shared",
                )
                nc.sync.dma_start(
                    agi.rearrange("(k p) c -> p k c", p=P), own_pay[:]
                )
                nc.gpsimd.collective_compute(
                    "AllGather",
                    mybir.AluOpType.bypass,
                    replica_groups=[list(range(NCORE))],
                    ins=[agi.opt()],
                    outs=[ago.opt()],
                )

                # ---- (B) s -> w_rep (=e^{.9s}) and s_rep per head (overlaps AG)
                w_rep = layerbuf.tile([P, h, ROWS], dt.bfloat16, tag="w_rep")
                s_rep = layerbuf.tile([P, h, ROWS], dt.bfloat16, tag="s_rep")
                for hh in range(h):
                    ps_row = sps.tile([1, ROWS], dt.float32, tag="ps_row")
                    for ft in range(nft):
                        fr = min(P, fin - ft * P)
                        nc.tensor.matmul(
                            ps_row[:],
                            ws_sb[li][ft][:, hh : hh + 1],
                            xcur[ft][0:fr, :],
                            start=(ft == 0),
                            stop=(ft == nft - 1),
                        )
                    w_row = small.tile([1, ROWS], dt.bfloat16, tag="w_row")
                    nc.scalar.activation(w_row[:], ps_row[:], AF.Exp, scale=0.9)
                    nc.gpsimd.partition_broadcast(w_rep[:, hh, :], w_row[:])
                    s_row = small.tile([1, ROWS], dt.bfloat16, tag="s_row")
                    nc.scalar.copy(s_row[:], ps_row[:])
                    nc.gpsimd.partition_broadcast(s_rep[:, hh, :], s_row[:])

                # ---- per group: unpack, attention, epilogue
                xnext = xpool.tile([fout, ROWS], dt.float32, tag="xT", name=f"xnext{li}")
                unp = []
                for gi, (g0, g1) in enumerate(groups):
                    g = g1 - g0
                    cols = goff[gi + 1] - goff[gi]
                    # packed readback on both HWDGE queues (SP + ACT), then
                    # re-stride on-chip into the 33-stride matmul layout
                    # (values [0:dh], ones at 32 — denominator lands at psum
                    # partition 32; engines only read partition bases 0/32/64/96)
                    ps_ = dh + 2
                    cmpp = layerbuf.tile(
                        [P, JT, g * ps_], dt.bfloat16, tag=f"cmpp{gi}", name=f"cmpp{li}_{gi}"
                    )
                    for r in range(NCORE):
                        eng = nc.sync if r % 2 == 0 else nc.scalar
                        eng.dma_start(
                            cmpp[:, 4 * r : 4 * (r + 1), :],
                            ago[r].rearrange("(k p) c -> p k c", p=P)[
                                :, :, goff[gi] : goff[gi + 1]
                            ],
                        )
                    cmp_t = layerbuf.tile(
                        [P, JT, g, 33], dt.bfloat16, tag=f"cmp{gi}", name=f"cmp{li}_{gi}"
                    )
                    cmp_v = cmp_t.rearrange("p j a b -> p (j a) b")
                    nc.gpsimd.memset(cmp_v[:, :, dh:32], 0.0)
                    nc.gpsimd.memset(cmp_v[:, :, 32:33], 1.0)
                    nc.vector.tensor_copy(
                        cmp_v[:, :, 0:dh],
                        cmpp.rearrange("p j (a b) -> p (j a) b", b=ps_)[:, :, 0:dh],
                    )
                    cmpd = cmpp.rearrange("p j (a b) -> p j a b", b=ps_)
                    d_sb = layerbuf.tile([P, JT, g], dt.float32, tag=f"dsb{gi}", name=f"dsb{li}_{gi}")
                    nc.vector.tensor_add(
                        d_sb[:], cmpd[:, :, :, dh], cmpd[:, :, :, dh + 1]
                    )
                    if debug_taps and li == 1 and gi == 0:
                        nc.sync.dma_start(dbg_d[:, :, 0:g], d_sb[:])
                    v_sb = layerbuf.tile([P, JT, g], dt.float32, tag=f"vsb{gi}", name=f"vsb{li}_{gi}")
                    vp_sb = layerbuf.tile([P, JT, g], dt.float32, tag=f"vpsb{gi}", name=f"vpsb{li}_{gi}")
                    nc.scalar.activation(v_sb[:], d_sb[:], AF.Exp)
                    nc.scalar.activation(vp_sb[:], d_sb[:], AF.Exp, scale=0.1)
                    unp.append((cmp_t, d_sb, v_sb, vp_sb))

                for gi, (g0, g1) in enumerate(groups):
                    g = g1 - g0
                    cmp_t, d_sb, v_sb, vp_sb = unp[gi]
                    # attention
                    att_acc = []
                    for k in range(g):
                        att_t = attps.tile([33, ROWS], dt.float32, tag="att", name=f"att{li}_{gi}_{k}")
                        att_acc.append(att_t)
                    for jt in range(JT):
                        # hybrid engine split: ~5/16 of j-tiles take the
                        # ACT (Lrelu+Exp) path, the rest the DVE
                        # (mul-max tensor_scalar) path; mask-mul on DVE.
                        act_path = (jt % 16) >= 11
                        p_jt = work.tile([P, g * ROWS], dt.bfloat16, tag="p_jt")
                        q_jt = work.tile([P, g * ROWS], dt.bfloat16, tag="q_jt")
                        l_jt = work.tile([P, g * ROWS], dt.bfloat16, tag="l_jt")
                        for k in range(g):
                            sl = slice(k * ROWS, (k + 1) * ROWS)
                            if act_path:
                                nc.scalar.activation(
                                    l_jt[:, sl],
                                    s_rep[:, g0 + k, :],
                                    AF.Lrelu,
                                    bias=d_sb[:, jt, k : k + 1],
                                    alpha=0.1,
                                )
                                nc.scalar.activation(q_jt[:, sl], l_jt[:, sl], AF.Exp)
                            else:
                                nc.vector.tensor_scalar(
                                    q_jt[:, sl],
                                    w_rep[:, g0 + k, :],
                                    v_sb[:, jt, k : k + 1],
                                    vp_sb[:, jt, k : k + 1],
                                    mybir.AluOpType.mult,
                                    mybir.AluOpType.max,
                                )
                            nc.vector.tensor_mul(
                                p_jt[:, sl], q_jt[:, sl], maskT[:, jt, :]
                            )
                        for k in range(g):
                            nc.tensor.matmul(
                                att_acc[k][:],
                                cmp_t[:, jt, k, :],
                                p_jt[:, k * ROWS : (k + 1) * ROWS],
                                start=(jt == 0),
                                stop=(jt == JT - 1),
                            )
                    # epilogue (all compute at partition base 0; DMA places
                    # results into xnext at head offsets)
                    for k in range(g):
                        hh = g0 + k
                        # epilogue reads att psum directly; denominator row is
                        # at psum partition 32 (legal ACT read base)
                        den0 = small.tile([1, ROWS], dt.float32, tag="den0")
                        nc.scalar.copy(den0[:], att_acc[k][32:33, :])
                        r_sb = small.tile([1, ROWS], dt.float32, tag="r_sb")
                        nc.vector.reciprocal_approx_fast(r_sb[:], den0[:])
                        rrep = small.tile([dh, ROWS], dt.float32, tag="rrep")
                        nc.gpsimd.partition_broadcast(rrep[:], r_sb[:])
                        ohead = small.tile([dh, ROWS], dt.float32, tag="ohead")
                        if elu:
                            x_sb = small.tile([dh, ROWS], dt.float32, tag="x_sb")
                            nc.vector.tensor_mul(x_sb[:], att_acc[k][0:dh, :], rrep[:])
                            e_sb = small.tile([dh, ROWS], dt.float32, tag="e_sb")
                            nc.scalar.activation(e_sb[:], x_sb[:], AF.Exp)
                            nc.vector._custom_dve(
                                ELUC, out=ohead[:], in0=x_sb[:], in1=e_sb[:], s0=-1.0
                            )
                        else:
                            nc.vector.tensor_mul(ohead[:], att_acc[k][0:dh, :], rrep[:])
                        (nc.sync if hh % 2 == 0 else nc.gpsimd).dma_start(
                            xnext[hh * dh : (hh + 1) * dh, :], ohead[:]
                        )

                if debug_taps:
                    nc.sync.dma_start(dbg_x[li][:], xnext[:])
                if is_last:
                    psum_final = small.tile([fout, 1], dt.float32, tag="pfin")
                    nc.vector.reduce_sum(
                        psum_final[:], xnext[:], axis=mybir.AxisListType.X
                    )
                    nc.sync.dma_start(pool_out[:], psum_final[:])
                else:
                    xcur = [xnext]

    nc.finalize()
    return nc


_NC_CACHE = None
_last_in_maps = None
_LAST_RES = None


def kernel(**inputs):
    global _NC_CACHE, _last_in_maps, _LAST_RES
    node_features = np.asarray(inputs["node_features"], dtype=np.float32)
    adj = np.asarray(inputs["adj_mat"], dtype=np.int32)
    fc_w = np.asarray(inputs["fc_w"], dtype=np.float32)
    fc_b = np.asarray(inputs["fc_b"], dtype=np.float32)

    x0T = np.ascontiguousarray(node_features.T)  # [256, N]
    adjTb = np.ascontiguousarray((adj.T != 0)).astype(BF16)  # [N(j), N(i)]

    wall = {}
    ws = {}
    for li, (fin, fout, h, dh, _e) in enumerate(CFG, start=1):
        W = np.asarray(inputs[f"W{li}"], dtype=np.float32)  # [h, fin, dh]
        a_src = np.asarray(inputs[f"a_src{li}"], dtype=np.float32)  # [h, dh]
        a_dst = np.asarray(inputs[f"a_dst{li}"], dtype=np.float32)
        wcat = W.transpose(1, 0, 2).reshape(fin, h * dh)
        wd = np.einsum("hfd,hd->fh", W, a_dst).astype(np.float32)
        wsrc = np.einsum("hfd,hd->fh", W, a_src).astype(np.float32)
        wall[li] = np.ascontiguousarray(np.concatenate([wcat, wd], axis=1))
        ws[li] = np.ascontiguousarray(wsrc)

    in_maps = []
    for c in range(NCORE):
        m = {
            "maskTd": np.ascontiguousarray(
                adjTb[:, c * ROWS : (c + 1) * ROWS]
            ).reshape(JT, P, ROWS),
            "x0T_own": np.ascontiguousarray(x0T[:, c * ROWS : (c + 1) * ROWS]),
        }
        for li in range(1, 6):
            m[f"wall{li}"] = wall[li]
            m[f"ws{li}"] = ws[li]
        in_maps.append(m)

    if _NC_CACHE is None:
        _NC_CACHE = build_kernel()
    nc = _NC_CACHE
    _last_in_maps = in_maps

    res = run_bass_kernel_spmd(nc, in_maps, list(range(NCORE)))
    _LAST_RES = res
    total = np.zeros((8,), dtype=np.float32)
    for c in range(NCORE):
        total += res.results[c]["pool_part"][:, 0]
    pooled = total / np.float32(N)
    out = pooled @ fc_w + fc_b
    return out.astype(np.float32)
